# revision 7
# baseline (speedup 1.0000x reference)
"""Binarized conv block (BinBlock) Trainium2 Bass kernel — fp16, 9-wave.

Reference computation (per image):
    xb    = sign(x);  alpha = mean|W| over (I,kh,kw)
    out   = conv2d(pad(xb,-1), alpha*sign(W)) + bias
    out   = out*gBN + (beta - mean*gBN) + x,   gBN = gamma/sqrt(var+eps)

Kernel algebra: let s = alpha*gBN, S2 = fp16(2s), b2 = bias*gBN + beta
- mean*gBN.  Activations binarize to b = (x>=0) in {0,1} (single-ALU-op
DVE sign; pad = 0), weights are S2[o]*sign(W) (fp16), so PE products are
{0, +/-S2} and PSUM column sums k*S2 are exact in fp32:
    psum = S2*sum(sign(W)*b) = s'*conv_int + 0.5*S2*sum(sign W)
    out  = psum + b2' + x,   b2' = b2 - 0.5*S2*sum_ct(sign W[o])  (host)

The residual x is NOT a matmul here (the v1 kernel spent 1 of its 10 PE
waves injecting x via a diag matmul; 9 waves/slot = ~10% less PE time).
Instead the epilogue is
    tmp = psum + b2'               (ScalarE activation, bias=b2')
    st  = tmp + x                  (DVE tensor_tensor, batched over m)
where the partition-aligned half of x ((img,ch) == (half,ch)) is read
straight from the raw-x tile and the crossed half is pre-copied into the
staging tiles by on-chip SBUF->SBUF DMAs (DMA is address-based, so the
partition relayout is free).  Only the crossed half is copied: a full
copy (6.4MB/core r+w) exceeds the ~368GB/s DMA-fabric budget that also
carries input and output (measured: full-copy variant runs 98us).
DVE adds are batched over m-ranges ((0,3),(3,5),(5,6)) because each DVE
op costs ~164ns fixed + ~0.52ns/elem: per-slot half-adds would put DVE
over the PE pace, batched ones leave ~0.2us/slot slack.  The last m=6
slots instead pre-copy BOTH halves (tiny) and run one fused DVE
scalar_tensor_tensor (st = (psum + b2') + st) per image so the tail
after the final matmul stays ~2us.

fp8 DoubleRow was re-examined and is definitively closed on this
toolchain: only P=128/M=128 ktile-major DR compiles (P=64 and M=64 fail
walrus ISA codegen; interleaved weights and byte-stride ktiles crash),
and with 64 output channels an M=128 DR instruction can only be
block-diagonal, which wastes exactly the 2x it would win.

I/O is fp16 end-to-end (host converts, rel err ~4e-4): DRAM layouts are
[pair][128][H*W] with partition = (img-in-pair)*64 + channel.  Inputs
stream on the sync HWDGE ring in row chunks; consts ride the scalar ring
so they don't delay the first chunk; outputs stage per image in SBUF and
drain in progressively finer cuts on the gpsimd/scalar rings.

Schedule notes:
  - Dep-free dummy matmuls on a memset tile warm the PE HAM clock gate
    (1.2 -> 2.4 GHz) during the ~11us startup window.
  - Next pair's input DMAs go out at (3,0); its sign ops and pre-copies
    are spread over slots (4,0)..(6,0) so their chunk-gated semaphore
    waits never head-of-line block a queue that carries epilogue work.

Measured on trn2 (8 cores, axon): v1 baseline 73.4us.  Note the chip
occasionally sits in a uniformly 1.2x slower clock state for a run or
two; compare runs via MATMUL median duration (~349ns fast, ~418 slow).
"""

import numpy as np

import concourse.bass as bass
import concourse.bacc as bacc
import concourse.tile as tile
import concourse.mybir as mybir
from concourse import bass_utils

F32 = mybir.dt.float32
F16 = mybir.dt.float16
U8 = mybir.dt.uint8

B, C, H, W = 32, 64, 112, 112
NCORES = 8
BSH = B // NCORES          # images per core
HWF = H * W                # 12544
HP = H + 2                 # 114 padded
SGW = HP * HP              # 12996
NB = 4 * W                 # 448 (one PSUM bank: 512 fp32)
NSLOT = 14                 # (m,j) slots per image
BN_EPS = 1e-5

ACT_ID = mybir.ActivationFunctionType.Identity
OP_GE = mybir.AluOpType.is_ge
OP_SUB = mybir.AluOpType.subtract
OP_MULT = mybir.AluOpType.mult
OP_ADD = mybir.AluOpType.add


def build_kernel_body(tc, out_d, x_d, cs_d):
    nc = tc.nc
    with (
        tc.tile_pool(name="const", bufs=1) as constp,
        tc.tile_pool(name="warmup", bufs=1) as warmupp,
        tc.tile_pool(name="xraw", bufs=2) as xrawp,
        tc.tile_pool(name="sign", bufs=2) as signp,
        tc.tile_pool(name="stage", bufs=4) as stagep,
        tc.tile_pool(name="tmp", bufs=6) as tmpp,
        tc.tile_pool(name="psum", bufs=8, space="PSUM") as psump,
    ):
        # consts in one byte tile; DMA on the scalar ring so the sync ring's
        # first x chunk is not queued behind it
        ct = constp.tile([128, 1156], U8)
        nc.scalar.dma_start(ct[:], cs_d[:])
        ws_t = ct[:, 0:1152].bitcast(F16)     # S2*sign(W)^T  [128, 576]
        b2_t = ct[:, 1152:1156].bitcast(F32)  # b2'           [128, 1]

        # PE warm-up: dep-free dummy matmuls keep the HAM activity monitor
        # busy during startup so the first real matmuls run at full clock
        wm = warmupp.tile([64, 520], F16)
        nc.gpsimd.memset(wm[:], 0.5)
        wps = psump.tile([128, NB], F32, name="ps_warm", tag="ps")
        for _ in range(6):
            nc.tensor.matmul(
                wps[0:8, :], wm[:, 512:520], wm[:, 0:448],
                start=True, stop=True, skip_group_check=True,
            )

        CHUNKS = ((0, 12), (12, 20), (20, 48), (48, 80), (80, H))
        # output cuts: slot ranges emitted at the given (m, j)
        OUT_CUTS = {(2, 1): (0, 6), (4, 1): (6, 10), (6, 0): (10, 12),
                    (6, 1): (12, 13), "end": (13, 14)}
        # DVE-add batches (m-ranges) for m<6; m=6 uses fused stt per slot
        BATCHES = ((0, 3), (3, 5), (5, 6))

        def chunk_dma(p, xr, ci):
            ra, rb = CHUNKS[ci]
            nc.sync.dma_start(xr[:, ra * W : rb * W], x_d[p, :, ra * W : rb * W])

        def chunk_sign(xr, sg3, ci):
            # binarize one row chunk: b = (x >= 0) in {0, 1}
            ra, rb = CHUNKS[ci]
            xr3 = xr[:].rearrange("p (h w) -> p h w", w=W)
            nc.vector.tensor_scalar(
                sg3[:, 1 + ra : 1 + rb, 1 : HP - 1],
                xr3[:, ra:rb, :],
                0.0,
                None,
                OP_GE,
            )

        def load_pair_dmas(p):
            xr = xrawp.tile([128, HWF], F16, name=f"xr_{p}", tag="xr")
            sg = signp.tile([128, SGW], F16, name=f"sg_{p}", tag="sg")
            sts = [
                stagep.tile([128, NSLOT * NB], F16, name=f"st_p{p}i{ih}", tag="st")
                for ih in range(2)
            ]
            sg3 = sg[:].rearrange("p (h w) -> p h w", w=HP)
            for ci in range(len(CHUNKS)):
                chunk_dma(p, xr, ci)
            # pad border b=0 (== sign -1 under the {0,1} encoding)
            nc.gpsimd.memset(sg3[:, 0, :], 0.0)
            nc.gpsimd.memset(sg3[:, HP - 1, :], 0.0)
            nc.gpsimd.memset(sg3[:, 1 : HP - 1, 0], 0.0)
            nc.gpsimd.memset(sg3[:, 1 : HP - 1, HP - 1], 0.0)
            return xr, sg, sg3, sts

        def precopy(xr, sts, ma, mb, aligned=False):
            # x -> staging tiles via on-chip DMA.  Image ih's x lives at
            # partitions ih*64+c; its hf=(1-ih) ("crossed") output half
            # stages at partitions (1-ih)*64+c, which only DMA can reach.
            # blk = 4m + 2hf + j.  aligned=True copies the hf==ih half
            # instead (used for m=6 so its epilogue is one fused DVE op).
            xr5 = xr[:].rearrange("p (m h j w) -> p m h j w", h=2, j=2, w=NB)
            for ih in range(2):
                st6 = sts[ih][:].rearrange("p (m j w) -> p m j w", j=2, w=NB)
                hf = ih if aligned else 1 - ih
                eng = nc.gpsimd if (ih == 0) else nc.scalar
                eng.dma_start(
                    st6[hf * 64 : hf * 64 + 64, ma:mb, :, :],
                    xr5[ih * 64 : ih * 64 + 64, ma:mb, hf, :, :],
                )

        def batch_add(xr, sts, tmps, ma, mb):
            # st[...] += tmp (aligned half also += x straight from xr)
            n = mb - ma
            xr5 = xr[:].rearrange("p (m h j w) -> p m h j w", h=2, j=2, w=NB)
            for ih in range(2):
                st6 = sts[ih][:].rearrange("p (m j w) -> p m j w", j=2, w=NB)
                tm6 = tmps[ih][:].rearrange("p (m j w) -> p m j w", j=2, w=NB)
                al, cr = ih, 1 - ih  # aligned hf == ih
                # aligned: st = tmp + x (x read in place from xr)
                nc.vector.tensor_tensor(
                    st6[al * 64 : al * 64 + 64, ma:mb, :, :],
                    tm6[al * 64 : al * 64 + 64, 0:n, :, :],
                    xr5[ih * 64 : ih * 64 + 64, ma:mb, al, :, :],
                    OP_ADD,
                )
                # crossed: st(=precopied x) += tmp, in place
                nc.vector.tensor_tensor(
                    st6[cr * 64 : cr * 64 + 64, ma:mb, :, :],
                    tm6[cr * 64 : cr * 64 + 64, 0:n, :, :],
                    st6[cr * 64 : cr * 64 + 64, ma:mb, :, :],
                    OP_ADD,
                )

        # prologue: pair 0 loads + signs up-front (pre-copies are emitted
        # just-in-time inside the slot loop so their chunk-gated waits never
        # head-of-line block the scalar/gpsimd queues ahead of epilogue work)
        pro = {}
        xr0, sg0, sg30, sts0 = load_pair_dmas(0)
        for ci in range(len(CHUNKS)):
            chunk_sign(xr0, sg30, ci)
        pro[0] = (xr0, sg0, sg30, sts0)

        for p in range(BSH // 2):  # image pairs; image 2p -> partitions 0:64
            xr, sg, sg3, sts = pro.pop(p)
            nxt = None
            tmps = None
            for m in range(7):
                bat = next(((a, b) for (a, b) in BATCHES if a <= m < b), None)
                for j in range(2):
                    # this pair's deferred last sign (emitted here so the DVE
                    # never waits on it; see next-pair block below)
                    if p > 0 and (m, j) == (0, 0):
                        chunk_sign(xr, sg3, 4)
                    # own x pre-copies, just-in-time (gates already landed)
                    if (m, j) == (1, 0):
                        precopy(xr, sts, 0, 3)
                    elif (m, j) == (3, 0):
                        precopy(xr, sts, 3, 5)
                    elif (m, j) == (4, 0):
                        precopy(xr, sts, 5, 7)
                    elif (m, j) == (5, 0):
                        precopy(xr, sts, 6, 7, aligned=True)
                    if m < 6 and (m, j) == (bat[0], 0):
                        blen = bat[1] - bat[0]
                        tmps = [
                            tmpp.tile([128, blen * 2 * NB], F16,
                                      name=f"tp_p{p}b{bat[0]}i{ih}", tag="tp")
                            for ih in range(2)
                        ]
                    psb = [
                        psump.tile(
                            [128, NB], F32, name=f"ps_p{p}m{m}j{j}i{ih}", tag="ps"
                        )
                        for ih in range(2)
                    ]
                    # 9 conv taps, round-robin over the 4 array quadrants
                    for pos in range(9):
                        dh, dw = divmod(pos, 3)
                        for q in range(4):
                            ih, hf = divmod(q, 2)
                            blk = 4 * m + 2 * hf + j
                            r0 = 4 * blk + dh
                            nc.tensor.matmul(
                                psb[ih][64 * hf : 64 * hf + 64, :],
                                ws_t[64 * ih : 64 * ih + 64, 64 * pos : 64 * pos + 64],
                                sg3[64 * ih : 64 * ih + 64, r0 : r0 + 4, dw : dw + W],
                                start=(pos == 0),
                                stop=(pos == 8),
                                skip_group_check=True,
                            )
                    if m < 6:
                        # epilogue part 1: tmp = psum + b2' (ScalarE)
                        for ih in range(2):
                            lo = (2 * (m - bat[0]) + j) * NB
                            nc.scalar.activation(
                                tmps[ih][:, lo : lo + NB], psb[ih][:, :],
                                ACT_ID, bias=b2_t[:, 0:1],
                            )
                        # epilogue part 2 at batch end: batched DVE adds
                        if (m, j) == (bat[1] - 1, 1):
                            batch_add(xr, sts, tmps, bat[0], bat[1])
                    else:
                        # m=6: both halves of x are pre-copied into st; one
                        # fused DVE op per image: st = (psum + b2') + st
                        for ih in range(2):
                            dst = sts[ih][:, (2 * m + j) * NB : (2 * m + j + 1) * NB]
                            nc.vector.scalar_tensor_tensor(
                                dst, psb[ih][:, :], b2_t[:, 0:1], dst,
                                OP_ADD, OP_ADD,
                            )
                    # stream each image out in progressively finer DMA cuts
                    cut = OUT_CUTS.get((m, j))
                    if cut:
                        lo, hi = (c * NB for c in cut)
                        last = (m, j) == (6, 1)
                        for ih in range(2):
                            n = 2 * p + ih
                            eng = nc.scalar if (last and ih == 0) else nc.gpsimd
                            eng.dma_start(out_d[n, :, lo:hi], sts[ih][:, lo:hi])
                    # next pair: DMAs at (3,1); signs spread over (5,1)..(6,1)
                    # (plus one deferred to the pair's own (0,0)) so their
                    # chunk-gated waits are satisfied before the DVE queue
                    # reaches them
                    if p + 1 < BSH // 2:
                        if (m, j) == (3, 1):
                            nxt = load_pair_dmas(p + 1)
                        elif (m, j) == (5, 1):
                            chunk_sign(nxt[0], nxt[2], 0)
                            chunk_sign(nxt[0], nxt[2], 1)
                        elif (m, j) == (6, 0):
                            chunk_sign(nxt[0], nxt[2], 2)
                        elif (m, j) == (6, 1):
                            chunk_sign(nxt[0], nxt[2], 3)
            # final small cut for this pair (slot 13) after the last stt
            lo, hi = (c * NB for c in OUT_CUTS["end"])
            for ih in range(2):
                n = 2 * p + ih
                eng = nc.gpsimd if ih == 0 else nc.scalar
                eng.dma_start(out_d[n, :, lo:hi], sts[ih][:, lo:hi])
            if nxt is not None:
                pro[p + 1] = nxt


def build_nc():
    nc = bacc.Bacc(trn_type="TRN2", debug=False, num_devices=NCORES)
    x_d = nc.dram_tensor("x", [BSH // 2, 128, HWF], F16, kind="ExternalInput")
    cs_d = nc.dram_tensor("consts", [128, 1156], U8, kind="ExternalInput")
    out_d = nc.dram_tensor("out", [BSH, 128, NSLOT * NB], F16, kind="ExternalOutput")
    with tile.TileContext(nc) as tc:
        build_kernel_body(tc, out_d, x_d, cs_d)
    nc.compile()
    return nc


def prep_consts(weight, bias, gamma, beta, run_mean, run_var):
    """Host-side constant prep (numpy, fp64 for the folding math)."""
    w = np.asarray(weight, np.float64)
    alpha = np.mean(np.abs(w), axis=(1, 2, 3))            # [O]
    g = np.asarray(gamma, np.float64) / np.sqrt(np.asarray(run_var, np.float64) + BN_EPS)
    s = alpha * g                                          # [O]
    b2 = np.asarray(bias, np.float64) * g + np.asarray(beta, np.float64) - np.asarray(
        run_mean, np.float64
    ) * g

    # lhsT layout [I(dup to 128), tap, O]; entries S2*sign(W), S2 = fp16(2s):
    # products with b in {0,1} are exactly {0, +/-S2}
    S2 = np.float16(2.0 * s).astype(np.float64)            # [O]
    wsign = np.sign(w)                                     # [O, I, 3, 3]
    wsc = wsign * S2[:, None, None, None]
    ws = wsc.transpose(1, 2, 3, 0).reshape(C, 9 * C)
    ws128 = np.concatenate([ws, ws], axis=0).astype(np.float16)

    # b2' absorbs the {0,1}-encoding correction: -0.5*S2*sum(sign W)
    b2p = b2 - 0.5 * S2 * wsign.sum(axis=(1, 2, 3))
    bi = np.concatenate([b2p, b2p]).astype(np.float32)[:, None]  # [128, 1]
    packed = np.concatenate(
        [
            np.ascontiguousarray(ws128).view(np.uint8),
            np.ascontiguousarray(bi).view(np.uint8),
        ],
        axis=1,
    )  # [128, 1156]
    return packed


_CACHE = {}


def kernel(x, weight, bias, gamma, beta, run_mean, run_var, _trace=False, _trace_kwargs=None):
    x = np.asarray(x)
    consts = prep_consts(weight, bias, gamma, beta, run_mean, run_var)
    # [core][pair][ih*64+c][h*w] in fp16
    x16 = np.ascontiguousarray(
        x.reshape(NCORES, BSH // 2, 128, HWF).astype(np.float16)
    )

    if "nc" not in _CACHE:
        _CACHE["nc"] = build_nc()
    nc = _CACHE["nc"]

    in_maps = [dict(x=x16[i], consts=consts) for i in range(NCORES)]
    res = bass_utils.run_bass_kernel_spmd(
        nc,
        in_maps,
        core_ids=list(range(NCORES)),
        trace=_trace,
        **(_trace_kwargs or {}),
    )
    outs = []
    for i in range(NCORES):
        o = np.asarray(res.results[i]["out"])  # [4, 128, 6272] fp16
        # partition=(hf,c), free=(m,j,r,w); row = m*16 + hf*8 + j*4 + r
        o = (
            o.reshape(BSH, 2, C, 7, 2, 4, W)
            .transpose(0, 2, 3, 1, 4, 5, 6)
            .reshape(BSH, C, H, W)
        )
        outs.append(o)
    out = np.concatenate(outs, axis=0).astype(np.float32)
    if _trace:
        kernel.last_results = res
    return out


# revision 9
# speedup vs baseline: 1.1468x; 1.1468x over previous
"""Binarized conv block (BinBlock) Trainium2 Bass kernel — fp16, 9-wave.

Reference computation (per image):
    xb    = sign(x);  alpha = mean|W| over (I,kh,kw)
    out   = conv2d(pad(xb,-1), alpha*sign(W)) + bias
    out   = out*gBN + (beta - mean*gBN) + x,   gBN = gamma/sqrt(var+eps)

Kernel algebra: let s = alpha*gBN, S2 = fp16(2s), b2 = bias*gBN + beta
- mean*gBN.  Activations binarize to b = (x>=0) in {0,1} (single-ALU-op
DVE sign; pad = 0), weights are S2[o]*sign(W) (fp16), so PE products are
{0, +/-S2} and PSUM column sums k*S2 are exact in fp32:
    psum = S2*sum(sign(W)*b) = s'*conv_int + 0.5*S2*sum(sign W)
    out  = psum + b2' + x,   b2' = b2 - 0.5*S2*sum_ct(sign W[o])  (host)

The residual x is NOT a matmul here (the v1 kernel spent 1 of its 10 PE
waves injecting x via a diag matmul; 9 waves/slot = ~10% less PE time).
Instead the epilogue is
    tmp = psum + b2'               (ScalarE activation, bias=b2')
    st  = tmp + x                  (DVE tensor_tensor, batched over m)
where the partition-aligned half of x ((img,ch) == (half,ch)) is read
straight from the raw-x tile and the crossed half is pre-copied into the
staging tiles by on-chip SBUF->SBUF DMAs (DMA is address-based, so the
partition relayout is free).  Only the crossed half is copied: a full
copy (6.4MB/core r+w) exceeds the ~368GB/s DMA-fabric budget that also
carries input and output (measured: full-copy variant runs 98us).
DVE adds are batched over m-ranges ((0,3),(3,5),(5,6)) because each DVE
op costs ~164ns fixed + ~0.52ns/elem: per-slot half-adds would put DVE
over the PE pace, batched ones leave ~0.2us/slot slack.  The last m=6
slots instead pre-copy BOTH halves (tiny) and run one fused DVE
scalar_tensor_tensor (st = (psum + b2') + st) per image so the tail
after the final matmul stays ~2us.

fp8 DoubleRow was re-examined and is definitively closed on this
toolchain: only P=128/M=128 ktile-major DR compiles (P=64 and M=64 fail
walrus ISA codegen; interleaved weights and byte-stride ktiles crash),
and with 64 output channels an M=128 DR instruction can only be
block-diagonal, which wastes exactly the 2x it would win.

I/O is fp16 end-to-end (host converts, rel err ~4e-4): DRAM layouts are
[pair][128][H*W] with partition = (img-in-pair)*64 + channel.  Inputs
stream on the sync HWDGE ring in row chunks; consts ride the scalar ring
so they don't delay the first chunk; outputs stage per image in SBUF and
drain in progressively finer cuts on the gpsimd/scalar rings.

Schedule notes:
  - Dep-free dummy matmuls on a memset tile warm the PE HAM clock gate
    (1.2 -> 2.4 GHz) during the ~11us startup window.
  - Next pair's input DMAs go out at (3,0); its sign ops and pre-copies
    are spread over slots (4,0)..(6,0) so their chunk-gated semaphore
    waits never head-of-line block a queue that carries epilogue work.

Measured on trn2 (8 cores, axon): v1 baseline 73.4us.  Note the chip
occasionally sits in a uniformly 1.2x slower clock state for a run or
two; compare runs via MATMUL median duration (~349ns fast, ~418 slow).
"""

import numpy as np

import concourse.bass as bass
import concourse.bacc as bacc
import concourse.tile as tile
import concourse.mybir as mybir
from concourse import bass_utils

F32 = mybir.dt.float32
F16 = mybir.dt.float16
U8 = mybir.dt.uint8

B, C, H, W = 32, 64, 112, 112
NCORES = 8
BSH = B // NCORES          # images per core
HWF = H * W                # 12544
HP = H + 2                 # 114 padded
SGW = HP * HP              # 12996
NB = 4 * W                 # 448 (one PSUM bank: 512 fp32)
NSLOT = 14                 # (m,j) slots per image
BN_EPS = 1e-5

ACT_ID = mybir.ActivationFunctionType.Identity
OP_GE = mybir.AluOpType.is_ge
OP_SUB = mybir.AluOpType.subtract
OP_MULT = mybir.AluOpType.mult
OP_ADD = mybir.AluOpType.add


def build_kernel_body(tc, out_d, x_d, xc_d, cs_d):
    nc = tc.nc
    with (
        tc.tile_pool(name="const", bufs=1) as constp,
        tc.tile_pool(name="warmup", bufs=1) as warmupp,
        tc.tile_pool(name="xraw", bufs=2) as xrawp,
        tc.tile_pool(name="sign", bufs=2) as signp,
        tc.tile_pool(name="stage", bufs=4) as stagep,
        tc.tile_pool(name="tmp", bufs=6) as tmpp,
        tc.tile_pool(name="psum", bufs=8, space="PSUM") as psump,
    ):
        # consts in one byte tile; DMA on the scalar ring so the sync ring's
        # first x chunk is not queued behind it
        ct = constp.tile([128, 1156], U8)
        nc.scalar.dma_start(ct[:], cs_d[:])
        ws_t = ct[:, 0:1152].bitcast(F16)     # S2*sign(W)^T  [128, 576]
        b2_t = ct[:, 1152:1156].bitcast(F32)  # b2'           [128, 1]

        # PE warm-up: dep-free dummy matmuls keep the HAM activity monitor
        # busy during startup so the first real matmuls run at full clock
        wm = warmupp.tile([64, 520], F16)
        nc.gpsimd.memset(wm[:], 0.5)
        wps = psump.tile([128, NB], F32, name="ps_warm", tag="ps")
        for _ in range(6):
            nc.tensor.matmul(
                wps[0:8, :], wm[:, 512:520], wm[:, 0:448],
                start=True, stop=True, skip_group_check=True,
            )

        CHUNKS = ((0, 12), (12, 20), (20, 48), (48, 80), (80, H))
        # output cuts: slot ranges emitted at the given (m, j)
        OUT_CUTS = {(2, 1): (0, 6), (4, 1): (6, 10), (6, 0): (10, 12),
                    (6, 1): (12, 13), "end": (13, 14)}
        # DVE-add batches (m-ranges) for m<6; m=6 uses fused stt per slot
        BATCHES = ((0, 3), (3, 5), (5, 6))

        def chunk_dma(p, xr, ci):
            ra, rb = CHUNKS[ci]
            nc.sync.dma_start(xr[:, ra * W : rb * W], x_d[p, :, ra * W : rb * W])

        def chunk_sign(xr, sg3, ci):
            # binarize one row chunk: b = (x >= 0) in {0, 1}
            ra, rb = CHUNKS[ci]
            xr3 = xr[:].rearrange("p (h w) -> p h w", w=W)
            nc.vector.tensor_scalar(
                sg3[:, 1 + ra : 1 + rb, 1 : HP - 1],
                xr3[:, ra:rb, :],
                0.0,
                None,
                OP_GE,
            )

        def load_pair_dmas(p):
            xr = xrawp.tile([128, HWF], F16, name=f"xr_{p}", tag="xr")
            sg = signp.tile([128, SGW], F16, name=f"sg_{p}", tag="sg")
            sts = [
                stagep.tile([128, NSLOT * NB], F16, name=f"st_p{p}i{ih}", tag="st")
                for ih in range(2)
            ]
            sg3 = sg[:].rearrange("p (h w) -> p h w", w=HP)
            for ci in range(len(CHUNKS)):
                chunk_dma(p, xr, ci)
            # pad border b=0 (== sign -1 under the {0,1} encoding)
            nc.gpsimd.memset(sg3[:, 0, :], 0.0)
            nc.gpsimd.memset(sg3[:, HP - 1, :], 0.0)
            nc.gpsimd.memset(sg3[:, 1 : HP - 1, 0], 0.0)
            nc.gpsimd.memset(sg3[:, 1 : HP - 1, HP - 1], 0.0)
            return xr, sg, sg3, sts

        def precopy(p, sts, ma, mb, aligned=False):
            # crossed half of x -> staging tiles, straight from DRAM: xc_d
            # holds it pre-arranged slot-major on the host so each DMA is 64
            # partitions x one multi-KB contiguous run (an SBUF->SBUF
            # partition-crossing relayout would shatter into 896B
            # descriptors and crawl at ~50GB/s).  aligned=True pulls the
            # m=6 aligned half from xc_d's two extra slot columns.
            for ih in range(2):
                hf = ih if aligned else 1 - ih
                plo = hf * 64
                if aligned:
                    slo, shi = 14 * NB, 16 * NB
                    dlo, dhi = 12 * NB, 14 * NB
                else:
                    slo, shi = 2 * ma * NB, 2 * mb * NB
                    dlo, dhi = slo, shi
                nc.sync.dma_start(
                    sts[ih][plo : plo + 64, dlo:dhi],
                    xc_d[p, plo : plo + 64, slo:shi],
                )

        def batch_add(xr, sts, tmps, ma, mb):
            # st[...] += tmp (aligned half also += x straight from xr)
            n = mb - ma
            xr5 = xr[:].rearrange("p (m h j w) -> p m h j w", h=2, j=2, w=NB)
            for ih in range(2):
                st6 = sts[ih][:].rearrange("p (m j w) -> p m j w", j=2, w=NB)
                tm6 = tmps[ih][:].rearrange("p (m j w) -> p m j w", j=2, w=NB)
                al, cr = ih, 1 - ih  # aligned hf == ih
                # aligned: st = tmp + x (x read in place from xr)
                nc.vector.tensor_tensor(
                    st6[al * 64 : al * 64 + 64, ma:mb, :, :],
                    tm6[al * 64 : al * 64 + 64, 0:n, :, :],
                    xr5[ih * 64 : ih * 64 + 64, ma:mb, al, :, :],
                    OP_ADD,
                )
                # crossed: st(=precopied x) += tmp, in place
                nc.vector.tensor_tensor(
                    st6[cr * 64 : cr * 64 + 64, ma:mb, :, :],
                    tm6[cr * 64 : cr * 64 + 64, 0:n, :, :],
                    st6[cr * 64 : cr * 64 + 64, ma:mb, :, :],
                    OP_ADD,
                )

        # prologue: pair 0 loads + signs up-front (pre-copies are emitted
        # just-in-time inside the slot loop so their chunk-gated waits never
        # head-of-line block the scalar/gpsimd queues ahead of epilogue work)
        pro = {}
        xr0, sg0, sg30, sts0 = load_pair_dmas(0)
        for ci in range(len(CHUNKS)):
            chunk_sign(xr0, sg30, ci)
        pro[0] = (xr0, sg0, sg30, sts0)

        for p in range(BSH // 2):  # image pairs; image 2p -> partitions 0:64
            xr, sg, sg3, sts = pro.pop(p)
            nxt = None
            tmps = None
            for m in range(7):
                bat = next(((a, b) for (a, b) in BATCHES if a <= m < b), None)
                for j in range(2):
                    # this pair's deferred last sign (emitted here so the DVE
                    # never waits on it; see next-pair block below)
                    if p > 0 and (m, j) == (0, 0):
                        chunk_sign(xr, sg3, 4)
                    # own x pre-copies, just-in-time (gates already landed)
                    if (m, j) == (1, 0):
                        precopy(p, sts, 0, 3)
                    elif (m, j) == (3, 0):
                        precopy(p, sts, 3, 5)
                    elif (m, j) == (4, 0):
                        precopy(p, sts, 5, 7)
                    elif (m, j) == (5, 0):
                        precopy(p, sts, 6, 7, aligned=True)
                    if m < 6 and (m, j) == (bat[0], 0):
                        blen = bat[1] - bat[0]
                        tmps = [
                            tmpp.tile([128, blen * 2 * NB], F16,
                                      name=f"tp_p{p}b{bat[0]}i{ih}", tag="tp")
                            for ih in range(2)
                        ]
                    psb = [
                        psump.tile(
                            [128, NB], F32, name=f"ps_p{p}m{m}j{j}i{ih}", tag="ps"
                        )
                        for ih in range(2)
                    ]
                    # 9 conv taps, round-robin over the 4 array quadrants
                    for pos in range(9):
                        dh, dw = divmod(pos, 3)
                        for q in range(4):
                            ih, hf = divmod(q, 2)
                            blk = 4 * m + 2 * hf + j
                            r0 = 4 * blk + dh
                            nc.tensor.matmul(
                                psb[ih][64 * hf : 64 * hf + 64, :],
                                ws_t[64 * ih : 64 * ih + 64, 64 * pos : 64 * pos + 64],
                                sg3[64 * ih : 64 * ih + 64, r0 : r0 + 4, dw : dw + W],
                                start=(pos == 0),
                                stop=(pos == 8),
                                skip_group_check=True,
                            )
                    if m < 6:
                        # epilogue part 1: tmp = psum + b2' (ScalarE)
                        for ih in range(2):
                            lo = (2 * (m - bat[0]) + j) * NB
                            nc.scalar.activation(
                                tmps[ih][:, lo : lo + NB], psb[ih][:, :],
                                ACT_ID, bias=b2_t[:, 0:1],
                            )
                        # epilogue part 2 at batch end: batched DVE adds
                        if (m, j) == (bat[1] - 1, 1):
                            batch_add(xr, sts, tmps, bat[0], bat[1])
                    else:
                        # m=6: both halves of x are pre-copied into st; one
                        # fused DVE op per image: st = (psum + b2') + st
                        for ih in range(2):
                            dst = sts[ih][:, (2 * m + j) * NB : (2 * m + j + 1) * NB]
                            nc.vector.scalar_tensor_tensor(
                                dst, psb[ih][:, :], b2_t[:, 0:1], dst,
                                OP_ADD, OP_ADD,
                            )
                    # stream each image out in progressively finer DMA cuts
                    cut = OUT_CUTS.get((m, j))
                    if cut:
                        lo, hi = (c * NB for c in cut)
                        last = (m, j) == (6, 1)
                        for ih in range(2):
                            n = 2 * p + ih
                            eng = nc.scalar if (last and ih == 0) else nc.gpsimd
                            eng.dma_start(out_d[n, :, lo:hi], sts[ih][:, lo:hi])
                    # next pair: DMAs at (3,1); signs spread over (5,1)..(6,1)
                    # (plus one deferred to the pair's own (0,0)) so their
                    # chunk-gated waits are satisfied before the DVE queue
                    # reaches them
                    if p + 1 < BSH // 2:
                        if (m, j) == (3, 1):
                            nxt = load_pair_dmas(p + 1)
                        elif (m, j) == (5, 1):
                            chunk_sign(nxt[0], nxt[2], 0)
                            chunk_sign(nxt[0], nxt[2], 1)
                        elif (m, j) == (6, 0):
                            chunk_sign(nxt[0], nxt[2], 2)
                        elif (m, j) == (6, 1):
                            chunk_sign(nxt[0], nxt[2], 3)
            # final small cut for this pair (slot 13) after the last stt
            lo, hi = (c * NB for c in OUT_CUTS["end"])
            for ih in range(2):
                n = 2 * p + ih
                eng = nc.gpsimd if ih == 0 else nc.scalar
                eng.dma_start(out_d[n, :, lo:hi], sts[ih][:, lo:hi])
            if nxt is not None:
                pro[p + 1] = nxt


def build_nc():
    nc = bacc.Bacc(trn_type="TRN2", debug=False, num_devices=NCORES)
    x_d = nc.dram_tensor("x", [BSH // 2, 128, HWF], F16, kind="ExternalInput")
    xc_d = nc.dram_tensor("xc", [BSH // 2, 128, 16 * NB], F16, kind="ExternalInput")
    cs_d = nc.dram_tensor("consts", [128, 1156], U8, kind="ExternalInput")
    out_d = nc.dram_tensor("out", [BSH, 128, NSLOT * NB], F16, kind="ExternalOutput")
    with tile.TileContext(nc) as tc:
        build_kernel_body(tc, out_d, x_d, xc_d, cs_d)
    nc.compile()
    return nc


def prep_consts(weight, bias, gamma, beta, run_mean, run_var):
    """Host-side constant prep (numpy, fp64 for the folding math)."""
    w = np.asarray(weight, np.float64)
    alpha = np.mean(np.abs(w), axis=(1, 2, 3))            # [O]
    g = np.asarray(gamma, np.float64) / np.sqrt(np.asarray(run_var, np.float64) + BN_EPS)
    s = alpha * g                                          # [O]
    b2 = np.asarray(bias, np.float64) * g + np.asarray(beta, np.float64) - np.asarray(
        run_mean, np.float64
    ) * g

    # lhsT layout [I(dup to 128), tap, O]; entries S2*sign(W), S2 = fp16(2s):
    # products with b in {0,1} are exactly {0, +/-S2}
    S2 = np.float16(2.0 * s).astype(np.float64)            # [O]
    wsign = np.sign(w)                                     # [O, I, 3, 3]
    wsc = wsign * S2[:, None, None, None]
    ws = wsc.transpose(1, 2, 3, 0).reshape(C, 9 * C)
    ws128 = np.concatenate([ws, ws], axis=0).astype(np.float16)

    # b2' absorbs the {0,1}-encoding correction: -0.5*S2*sum(sign W)
    b2p = b2 - 0.5 * S2 * wsign.sum(axis=(1, 2, 3))
    bi = np.concatenate([b2p, b2p]).astype(np.float32)[:, None]  # [128, 1]
    packed = np.concatenate(
        [
            np.ascontiguousarray(ws128).view(np.uint8),
            np.ascontiguousarray(bi).view(np.uint8),
        ],
        axis=1,
    )  # [128, 1156]
    return packed


_CACHE = {}


def kernel(x, weight, bias, gamma, beta, run_mean, run_var, _trace=False, _trace_kwargs=None):
    x = np.asarray(x)
    consts = prep_consts(weight, bias, gamma, beta, run_mean, run_var)
    # [core][pair][ih*64+c][h*w] in fp16
    x16 = np.ascontiguousarray(
        x.reshape(NCORES, BSH // 2, 128, HWF).astype(np.float16)
    )
    # crossed-half of x pre-arranged slot-major (+ aligned m=6 in the two
    # extra columns) so the kernel's staging pre-copies are big contiguous
    # DRAM->SBUF descriptors
    xb = x16.reshape(NCORES, BSH // 2, 2, 64, 28, NB)  # [core,pair,ih,c,blk,k]
    xc = np.empty((NCORES, BSH // 2, 128, 16 * NB), np.float16)
    x5 = xb.reshape(NCORES, BSH // 2, 2, 64, 7, 2, 2, NB)  # [...,m,hf,j,k]
    # img0 (ih=0) crossed hf=1 -> partitions 64:128, slots (2m+j)
    xc[:, :, 64:128, : 14 * NB] = x5[:, :, 0, :, :, 1, :, :].reshape(
        NCORES, BSH // 2, 64, 14 * NB)
    # img1 (ih=1) crossed hf=0 -> partitions 0:64
    xc[:, :, 0:64, : 14 * NB] = x5[:, :, 1, :, :, 0, :, :].reshape(
        NCORES, BSH // 2, 64, 14 * NB)
    # aligned m=6: img0 hf0 (blk 24+j) -> parts 0:64; img1 hf1 (blk 26+j)
    xc[:, :, 0:64, 14 * NB :] = x5[:, :, 0, :, 6, 0, :, :].reshape(
        NCORES, BSH // 2, 64, 2 * NB)
    xc[:, :, 64:128, 14 * NB :] = x5[:, :, 1, :, 6, 1, :, :].reshape(
        NCORES, BSH // 2, 64, 2 * NB)
    xc = np.ascontiguousarray(xc)

    if "nc" not in _CACHE:
        _CACHE["nc"] = build_nc()
    nc = _CACHE["nc"]

    in_maps = [dict(x=x16[i], xc=xc[i], consts=consts) for i in range(NCORES)]
    res = bass_utils.run_bass_kernel_spmd(
        nc,
        in_maps,
        core_ids=list(range(NCORES)),
        trace=_trace,
        **(_trace_kwargs or {}),
    )
    outs = []
    for i in range(NCORES):
        o = np.asarray(res.results[i]["out"])  # [4, 128, 6272] fp16
        # partition=(hf,c), free=(m,j,r,w); row = m*16 + hf*8 + j*4 + r
        o = (
            o.reshape(BSH, 2, C, 7, 2, 4, W)
            .transpose(0, 2, 3, 1, 4, 5, 6)
            .reshape(BSH, C, H, W)
        )
        outs.append(o)
    out = np.concatenate(outs, axis=0).astype(np.float32)
    if _trace:
        kernel.last_results = res
    return out
